# revision 13
# baseline (speedup 1.0000x reference)
"""Euclidean distance layer on 8 Trainium2 NeuronCores.

out[b, o] = || x[b, :] - weight[:, o] ||_2
x: [512, 256] f32, weight: [256, 1024] f32 -> out: [512, 1024] f32

Sharding: tensor-parallel over output features (8 x 128 columns per core).

Per core:  dist^2 = -2 * ( x@w_loc  - 0.5*||w_loc||^2  - 0.5*||x||^2 )
  - x@w_loc on the PE in bf16 into one fp32 PSUM bank [128, 4x128]
    (bf16 operand rounding is ~2e-5 relative on dist: the x.w term is small
    vs the norm terms, which stay fp32)
  - ||w||^2: bf16 squares (DVE) -> ones-column reduce matmul -> broadcast
    row, folded in with a single K=1 bf16 matmul into the same bank
  - ||x||^2: computed in fp32 on the DVE from natural-layout x via fused
    square+reduce (tensor_tensor_reduce) -> per-partition column, added to
    PSUM with tensor_scalar_add
  - one ACT op: out = sqrt(-2 * psum)
Raw bacc: manual semaphores; no Tile event-sem traffic or exit barriers.
Host work is layout only: transpose x, slice w, concat output slices.
"""

from contextlib import ExitStack

import numpy as np

B = 512      # batch
K = 256      # inputSize (contraction dim)
NOUT = 1024  # outputSize
NCORES = 8
NLOC = NOUT // NCORES  # 128 output features per core
P = 128                # partitions
KT = K // P            # 2 contraction chunks
MT = B // P            # 4 batch tiles
KN = K                 # natural-layout row length

_NC = None  # cached compiled Bass program (same SPMD program on all cores)


def _build():
    import concourse.bass as bass
    from concourse import bacc, mybir

    f32 = mybir.dt.float32
    bf16 = mybir.dt.bfloat16
    Sqrt = mybir.ActivationFunctionType.Sqrt
    ts = bass.ts

    nc = bacc.Bacc(
        "TRN2", target_bir_lowering=False, debug=False, num_devices=NCORES
    )

    xt = nc.dram_tensor("xt", [K, B], f32, kind="ExternalInput")
    xn = nc.dram_tensor("xn", [B, K], f32, kind="ExternalInput")
    wl = nc.dram_tensor("wl", [K, NLOC], f32, kind="ExternalInput")
    out = nc.dram_tensor("out", [B, NLOC], f32, kind="ExternalOutput")

    with ExitStack() as ctx:
        e = ctx.enter_context
        xt_sb = [e(nc.sbuf_tensor(f"xts{k}", [P, B], f32)) for k in range(KT)]
        xt_b = [e(nc.sbuf_tensor(f"xtb{k}", [P, B], bf16)) for k in range(KT)]
        wl_sb = [e(nc.sbuf_tensor(f"wls{k}", [P, NLOC], f32)) for k in range(KT)]
        wl_b = [e(nc.sbuf_tensor(f"wlb{k}", [P, NLOC], bf16)) for k in range(KT)]
        wlsq = [e(nc.sbuf_tensor(f"wlsq{k}", [P, NLOC], bf16)) for k in range(KT)]
        xn_sb = [e(nc.sbuf_tensor(f"xns{m}", [P, KN], f32)) for m in range(MT)]
        xsq_scr = [e(nc.sbuf_tensor(f"xsqs{m}", [P, KN], f32)) for m in range(MT)]
        xsq_col = [e(nc.sbuf_tensor(f"xsqc{m}", [P, 1], f32)) for m in range(MT)]
        neg_q = e(nc.sbuf_tensor("neg_q", [P, 2], bf16))
        ones_m = e(nc.sbuf_tensor("ones_m", [2, P], bf16))
        wsq_row4 = e(nc.sbuf_tensor("wsq_row4", [2, MT, NLOC], bf16))
        out_sb = e(nc.sbuf_tensor("out_sb", [P, MT, NLOC], f32))
        actwarm = e(nc.sbuf_tensor("actwarm", [1, 1], f32))

        ps_w = e(nc.psum_tensor("ps_w", [2, NLOC], f32))     # 2 rows of -0.25*||w||^2
        ps_all = e(nc.psum_tensor("ps_all", [P, MT, NLOC], f32))  # one bank

        s_xt = [e(nc.semaphore(f"s_xt{k}")) for k in range(KT)]
        s_wl = [e(nc.semaphore(f"s_wl{k}")) for k in range(KT)]
        s_xn = [e(nc.semaphore(f"s_xn{m}")) for m in range(MT)]
        s_k = [e(nc.semaphore(f"s_k{k}")) for k in range(KT)]  # casts chunk k
        s_sq = e(nc.semaphore("s_sq"))      # 2 = both wlsq done
        s_col = e(nc.semaphore("s_col"))    # m+1 = xsq_col[m] ready
        s_mm = e(nc.semaphore("s_mm"))      # 1 = wsq reduce, 2 = all matmuls
        s_brd = e(nc.semaphore("s_brd"))    # 1 = wsq_row4 broadcast ready
        s_add = e(nc.semaphore("s_add"))    # 4 = all xsq adds done
        s_sqrt = e(nc.semaphore("s_sqrt"))  # 1 = out_sb written
        s_out = e(nc.semaphore("s_out"))    # 32 = both output DMAs landed
        block = e(nc.Block())

        @block.sync
        def _(sync):
            sync.dma_start(
                out=xt_sb[0][:, :], in_=xt[0:P, :]
            ).then_inc(s_xt[0], 16)
            sync.dma_start(
                out=wl_sb[0][:, :], in_=wl[0:P, :]
            ).then_inc(s_wl[0], 16)
            for m in (2, 3):
                sync.dma_start(
                    out=xn_sb[m][:, :], in_=xn[m * P : (m + 1) * P, :]
                ).then_inc(s_xn[m], 16)
            sync.wait_ge(s_sqrt, 1)
            sync.dma_start(
                out=out[0 : 2 * P, :].rearrange("(m p) o -> p m o", p=P),
                in_=out_sb[:, 0:2, :],
            ).then_inc(s_out, 16)
            sync.wait_ge(s_out, 32)

        @block.scalar
        def _(scalar):
            scalar.dma_start(
                out=xt_sb[1][:, :], in_=xt[P : 2 * P, :]
            ).then_inc(s_xt[1], 16)
            scalar.dma_start(
                out=wl_sb[1][:, :], in_=wl[P : 2 * P, :]
            ).then_inc(s_wl[1], 16)
            for m in (0, 1):
                scalar.dma_start(
                    out=xn_sb[m][:, :], in_=xn[m * P : (m + 1) * P, :]
                ).then_inc(s_xn[m], 16)
            # prime the Sqrt ACT table while DMAs fly (scale=0 -> sqrt(0))
            scalar.activation(actwarm[:, :], actwarm[:, :], Sqrt, scale=0.0)
            scalar.wait_ge(s_add, MT)
            scalar.activation(
                out_sb[:, :, :], ps_all[:, :, :], Sqrt, scale=-2.0
            ).then_inc(s_sqrt)
            scalar.wait_ge(s_sqrt, 1)
            scalar.dma_start(
                out=out[2 * P : 4 * P, :].rearrange("(m p) o -> p m o", p=P),
                in_=out_sb[:, 2:4, :],
            ).then_inc(s_out, 16)

        @block.vector
        def _(vector):
            vector.memset(neg_q[:, :], -0.25)
            vector.memset(ones_m[:, :], 1.0)
            # bf16 matmul operands, most-urgent first
            vector.wait_ge(s_xt[0], 16)
            vector.tensor_copy(xt_b[0][:, :], xt_sb[0][:, :]).then_inc(s_k[0])
            vector.wait_ge(s_wl[0], 16)
            vector.tensor_copy(wl_b[0][:, :], wl_sb[0][:, :]).then_inc(s_k[0])
            vector.wait_ge(s_xt[1], 16)
            vector.tensor_copy(xt_b[1][:, :], xt_sb[1][:, :]).then_inc(s_k[1])
            vector.wait_ge(s_wl[1], 16)
            vector.tensor_copy(wl_b[1][:, :], wl_sb[1][:, :]).then_inc(s_k[1])
            # w squares for the ||w||^2 reduce
            vector.tensor_mul(
                wlsq[0][:, :], wl_sb[0][:, :], wl_sb[0][:, :]
            ).then_inc(s_sq)
            vector.tensor_mul(
                wlsq[1][:, :], wl_sb[1][:, :], wl_sb[1][:, :]
            ).then_inc(s_sq)
            # -0.5*||x||^2 per-partition columns (fp32 square + free-dim reduce)
            for m in range(MT):
                vector.wait_ge(s_xn[m], 16)
                vector.tensor_mul(
                    xsq_scr[m][:, :], xn_sb[m][:, :], xn_sb[m][:, :]
                )
                vector.drain()
                vector.tensor_reduce(
                    xsq_col[m][:, :], xsq_scr[m][:, :],
                    axis=mybir.AxisListType.X, op=mybir.AluOpType.add,
                )
                vector.drain()
                vector.tensor_scalar_mul(
                    xsq_col[m][:, :], xsq_col[m][:, :], -0.5
                ).then_inc(s_col)
            # broadcast -0.5*||w||^2 row across the 4 m-slices
            vector.wait_ge(s_mm, 1)
            wsq_bcast_in = bass.AP(
                tensor=ps_w,
                offset=0,
                ap=[[NLOC, 2], [0, MT], [1, NLOC]],
            )
            vector.tensor_copy(wsq_row4[:, :, :], wsq_bcast_in).then_inc(s_brd)
            # add -0.5*||x||^2 into the PSUM bank after all matmuls
            vector.wait_ge(s_mm, 2)
            for m in range(MT):
                vector.wait_ge(s_col, m + 1)
                vector.tensor_scalar_add(
                    ps_all[:, m, :], ps_all[:, m, :], xsq_col[m][:, :]
                ).then_inc(s_add)

        @block.tensor
        def _(tensor):
            # main bf16 matmuls: one PSUM bank, single start on the first
            tensor.wait_ge(s_k[0], 2)
            for m in range(MT):
                tensor.matmul(
                    ps_all[:, m, :],
                    lhsT=xt_b[0][:, ts(m, P)],
                    rhs=wl_b[0][:, :],
                    start=(m == 0), stop=False,
                )
            # -0.5*||w||^2 reduce (separate bank)
            tensor.wait_ge(s_sq, 2)
            tensor.matmul(
                ps_w[:, :], lhsT=neg_q[:, :], rhs=wlsq[0][:, :],
                start=True, stop=False,
            )
            tensor.matmul(
                ps_w[:, :], lhsT=neg_q[:, :], rhs=wlsq[1][:, :],
                start=False, stop=True,
            ).then_inc(s_mm)  # = 1
            tensor.wait_ge(s_k[1], 2)
            for m in range(MT):
                tensor.matmul(
                    ps_all[:, m, :],
                    lhsT=xt_b[1][:, ts(m, P)],
                    rhs=wl_b[1][:, :],
                    start=False, stop=False,
                )
            # fold -0.5*||w||^2 into every m-slice with one K=1 matmul
            tensor.wait_ge(s_brd, 1)
            tensor.matmul(
                ps_all[:, :, :],
                lhsT=ones_m[:, :],
                rhs=wsq_row4[:, :, :],
                start=False, stop=True,
            ).then_inc(s_mm)  # = 2

    nc.compile()
    return nc


def _get_nc():
    global _NC
    if _NC is None:
        _NC = _build()
    return _NC


def _make_in_maps(x: np.ndarray, weight: np.ndarray):
    x = np.ascontiguousarray(x.astype(np.float32, copy=False))
    xt = np.ascontiguousarray(x.T)
    return [
        {
            "xt": xt,
            "xn": x,
            "wl": np.ascontiguousarray(weight[:, c * NLOC : (c + 1) * NLOC]),
        }
        for c in range(NCORES)
    ]


def run(x: np.ndarray, weight: np.ndarray, trace: bool = False):
    """Returns (full_output, BassKernelResults)."""
    from concourse.bass_utils import run_bass_kernel_spmd

    nc = _get_nc()
    res = run_bass_kernel_spmd(
        nc, _make_in_maps(x, weight), core_ids=list(range(NCORES)), trace=trace
    )
    full = np.concatenate(
        [res.results[c]["out"] for c in range(NCORES)], axis=1
    )
    return full, res


def kernel(x: np.ndarray, weight: np.ndarray) -> np.ndarray:
    return run(x, weight)[0]


# revision 15
# speedup vs baseline: 1.1593x; 1.1593x over previous
"""Euclidean distance layer on 8 Trainium2 NeuronCores.

out[b, o] = || x[b, :] - weight[:, o] ||_2
x: [512, 256] f32, weight: [256, 1024] f32 -> out: [512, 1024] f32

Sharding: tensor-parallel over output features (8 x 128 columns per core).

Per core:  dist^2 = -2 * ( x@w_loc - 0.5*||w_loc||^2 ) + ||x||^2
  - x@w_loc on the PE in bf16 into one fp32 PSUM bank [128, 4x128]
    (bf16 operand rounding is ~2e-5 relative on dist: the x.w term is small
    vs the norm terms, which stay fp32)
  - ||w||^2: bf16 squares (DVE) -> [-0.25] x2-column reduce matmul ->
    stride-0 broadcast row copy, folded in with K=2 bf16 matmuls
  - ||x||^2: ACT Square with accum_out on natural-layout x -> fp32
    per-partition column, consumed as the bias of the final sqrt
  - final: out = sqrt(-2 * psum + ||x||^2) on ACT (affine scale + bias)
Raw bacc: manual semaphores; no Tile event-sem traffic or exit barriers.
Host work is layout only: transpose x, slice w, concat output slices.
"""

from contextlib import ExitStack

import numpy as np

B = 512      # batch
K = 256      # inputSize (contraction dim)
NOUT = 1024  # outputSize
NCORES = 8
NLOC = NOUT // NCORES  # 128 output features per core
P = 128                # partitions
KT = K // P            # 2 contraction chunks
MT = B // P            # 4 batch tiles

_NC = None  # cached compiled Bass program (same SPMD program on all cores)


def _build():
    import concourse.bass as bass
    from concourse import bacc, mybir

    f32 = mybir.dt.float32
    bf16 = mybir.dt.bfloat16
    Sqrt = mybir.ActivationFunctionType.Sqrt
    Square = mybir.ActivationFunctionType.Square
    ts = bass.ts

    nc = bacc.Bacc(
        "TRN2", target_bir_lowering=False, debug=False, num_devices=NCORES
    )

    xt = nc.dram_tensor("xt", [K, B], f32, kind="ExternalInput")
    xn = nc.dram_tensor("xn", [B, K], f32, kind="ExternalInput")
    wl = nc.dram_tensor("wl", [K, NLOC], f32, kind="ExternalInput")
    out = nc.dram_tensor("out", [B, NLOC], f32, kind="ExternalOutput")

    with ExitStack() as ctx:
        e = ctx.enter_context
        # single-DMA staging tiles: [P, KT, free] with (c p) interleave
        xt_sb = e(nc.sbuf_tensor("xts", [P, KT, B], f32))
        wl_sb = e(nc.sbuf_tensor("wls", [P, KT, NLOC], f32))
        xn_sb = [e(nc.sbuf_tensor(f"xns{h}", [P, 2, K], f32)) for h in range(2)]
        xt_b = [e(nc.sbuf_tensor(f"xtb{k}", [P, B], bf16)) for k in range(KT)]
        wl_b = [e(nc.sbuf_tensor(f"wlb{k}", [P, NLOC], bf16)) for k in range(KT)]
        wlsq = [e(nc.sbuf_tensor(f"wlsq{k}", [P, NLOC], bf16)) for k in range(KT)]
        xsq_scr = e(nc.sbuf_tensor("xsqs", [P, MT, K], f32))
        xsq_col = e(nc.sbuf_tensor("xsqc", [P, MT], f32))
        neg_q = e(nc.sbuf_tensor("neg_q", [P, 2], bf16))
        ones_m = e(nc.sbuf_tensor("ones_m", [2, P], bf16))
        wsq_row4 = e(nc.sbuf_tensor("wsq_row4", [2, MT, NLOC], bf16))
        out_sb = e(nc.sbuf_tensor("out_sb", [P, MT, NLOC], f32))
        actwarm = e(nc.sbuf_tensor("actwarm", [1, 1], f32))

        ps_w = e(nc.psum_tensor("ps_w", [2, NLOC], f32))   # -0.25*||w||^2 x2
        ps_all = e(nc.psum_tensor("ps_all", [P, MT, NLOC], f32))  # one bank

        s_xt = e(nc.semaphore("s_xt"))
        s_wl = e(nc.semaphore("s_wl"))
        s_xn = [e(nc.semaphore(f"s_xn{h}")) for h in range(2)]
        s_k = [e(nc.semaphore(f"s_k{k}")) for k in range(KT)]
        s_sq = e(nc.semaphore("s_sq"))      # 2 = both wlsq done
        s_mm = e(nc.semaphore("s_mm"))      # 1 = wsq reduce, 2+m = aug m
        s_brd = e(nc.semaphore("s_brd"))    # 1 = wsq_row4 broadcast ready
        s_sqrt = e(nc.semaphore("s_sqrt"))  # m+1 = sqrt tile m in out_sb
        s_out = e(nc.semaphore("s_out"))    # 16 = output DMA landed
        block = e(nc.Block())

        @block.sync
        def _(sync):
            sync.dma_start(
                out=xt_sb[:, :, :],
                in_=xt[:, :].rearrange("(c p) b -> p c b", p=P),
            ).then_inc(s_xt, 16)
            sync.dma_start(
                out=xn_sb[1][:, :, :],
                in_=xn[2 * P : 4 * P, :].rearrange("(c p) k -> p c k", p=P),
            ).then_inc(s_xn[1], 16)
            sync.wait_ge(s_sqrt, MT)
            sync.dma_start(
                out=out[:, :].rearrange("(m p) o -> p m o", p=P),
                in_=out_sb[:, :, :],
            ).then_inc(s_out, 16)
            sync.wait_ge(s_out, 16)

        @block.scalar
        def _(scalar):
            scalar.dma_start(
                out=wl_sb[:, :, :],
                in_=wl[:, :].rearrange("(c p) o -> p c o", p=P),
            ).then_inc(s_wl, 16)
            scalar.dma_start(
                out=xn_sb[0][:, :, :],
                in_=xn[0 : 2 * P, :].rearrange("(c p) k -> p c k", p=P),
            ).then_inc(s_xn[0], 16)
            # prime the sqrt_and_others table (covers Square + Sqrt)
            scalar.activation(actwarm[:, :], actwarm[:, :], Sqrt, scale=0.0)
            # ||x||^2 columns via Square + free-dim accumulate
            for m in range(MT):
                scalar.wait_ge(s_xn[m // 2], 16)
                scalar.activation(
                    xsq_scr[:, m, :], xn_sb[m // 2][:, m % 2, :], Square,
                    accum_out=xsq_col[:, m : m + 1],
                )
            scalar.drain()  # ACT RAW: sqrts below read xsq_col
            for m in range(MT):
                scalar.wait_ge(s_mm, 2 + m)
                scalar.activation(
                    out_sb[:, m, :], ps_all[:, m, :], Sqrt,
                    bias=xsq_col[:, m : m + 1], scale=-2.0,
                ).then_inc(s_sqrt)

        @block.vector
        def _(vector):
            vector.memset(neg_q[:, :], -0.25)
            vector.memset(ones_m[:, :], 1.0)
            vector.wait_ge(s_wl, 16)
            vector.tensor_mul(wlsq[0][:, :], wl_sb[:, 0, :], wl_sb[:, 0, :])
            vector.tensor_mul(
                wlsq[1][:, :], wl_sb[:, 1, :], wl_sb[:, 1, :]
            ).then_inc(s_sq, 2)
            vector.tensor_copy(wl_b[0][:, :], wl_sb[:, 0, :])
            vector.tensor_copy(wl_b[1][:, :], wl_sb[:, 1, :])
            vector.wait_ge(s_xt, 16)
            vector.tensor_copy(xt_b[0][:, :], xt_sb[:, 0, :]).then_inc(s_k[0])
            vector.tensor_copy(xt_b[1][:, :], xt_sb[:, 1, :]).then_inc(s_k[1])
            vector.wait_ge(s_mm, 1)
            vector.tensor_copy(
                wsq_row4[:, :, :],
                bass.AP(tensor=ps_w, offset=0, ap=[[NLOC, 2], [0, MT], [1, NLOC]]),
            ).then_inc(s_brd)

        @block.tensor
        def _(tensor):
            # -0.25*||w||^2 reduce, two identical rows
            tensor.wait_ge(s_sq, 2)
            tensor.matmul(
                ps_w[:, :], lhsT=neg_q[:, :], rhs=wlsq[0][:, :],
                start=True, stop=False,
            )
            tensor.matmul(
                ps_w[:, :], lhsT=neg_q[:, :], rhs=wlsq[1][:, :],
                start=False, stop=True,
            ).then_inc(s_mm)  # = 1
            # main bf16 matmuls: one PSUM bank, single start on the first
            for k in range(KT):
                tensor.wait_ge(s_k[k], 1)
                for m in range(MT):
                    tensor.matmul(
                        ps_all[:, m, :],
                        lhsT=xt_b[k][:, ts(m, P)],
                        rhs=wl_b[k][:, :],
                        start=(k == 0 and m == 0), stop=False,
                        skip_group_check=True,
                    )
            # fold -0.5*||w||^2 per m-slice (K=2: two -0.25 rows)
            tensor.wait_ge(s_brd, 1)
            for m in range(MT):
                tensor.matmul(
                    ps_all[:, m, :],
                    lhsT=ones_m[:, :],
                    rhs=wsq_row4[:, m, :],
                    start=False, stop=True, skip_group_check=True,
                ).then_inc(s_mm)  # = 2 + m

    nc.compile()
    return nc


def _get_nc():
    global _NC
    if _NC is None:
        _NC = _build()
    return _NC


def _make_in_maps(x: np.ndarray, weight: np.ndarray):
    x = np.ascontiguousarray(x.astype(np.float32, copy=False))
    xt = np.ascontiguousarray(x.T)
    return [
        {
            "xt": xt,
            "xn": x,
            "wl": np.ascontiguousarray(weight[:, c * NLOC : (c + 1) * NLOC]),
        }
        for c in range(NCORES)
    ]


def run(x: np.ndarray, weight: np.ndarray, trace: bool = False):
    """Returns (full_output, BassKernelResults)."""
    from concourse.bass_utils import run_bass_kernel_spmd

    nc = _get_nc()
    res = run_bass_kernel_spmd(
        nc, _make_in_maps(x, weight), core_ids=list(range(NCORES)), trace=trace
    )
    full = np.concatenate(
        [res.results[c]["out"] for c in range(NCORES)], axis=1
    )
    return full, res


def kernel(x: np.ndarray, weight: np.ndarray) -> np.ndarray:
    return run(x, weight)[0]
